# revision 26
# baseline (speedup 1.0000x reference)
"""GCN block (3x GCNConv(128,128) + relu + global_mean_pool) on 8 trn2 cores.

v2 strategy (same graph partition by destination node as v1, re-engineered
around the measured bottlenecks: Q7 SWDGE descriptor generation, DVE
tensor_scalar S-builds, and their SBUF-port contention):

  - All device-side tensors are bf16 (PSUM accumulation stays fp32).
  - The one-hot scatter matrices S (static across layers!) are precomputed
    on the HOST in bf16 and streamed per block via HWDGE (nc.sync.dma_start)
    instead of being built per chunk on DVE.  DVE does nothing; pointwise
    work runs on the Scalar/ACT engine, so GpSimd's SWDGE descriptor
    generation no longer contends with DVE 2-port SBUF locks.
  - Self-loops are removed from the gather: each core keeps its own H shard
    resident in SBUF (hself) and applies the self-loop contribution as one
    extra chunk matmul against a host-built diagonal S (s_self).  This cuts
    ~12.5K gather descriptors per core per layer and shrinks bucket padding.
  - The bias is folded into the PE as a rank-1 matmul (ones[1,128]^T @
    b[1,128]) accumulating into the same PSUM as the H@W product, so
    bias+relu collapses into a single ACT activation(Relu) with bf16 cast.
  - Gathers go to 4 SWDGE queues (one per source quarter) so descriptor
    rings drain in parallel; blocks of 4 dst tiles per gather amortize the
    per-instruction overhead.

Per-core data layout (SPMD: same program, per-core arrays):
  - nodes split into 8 shards of 12544 (98 tiles of 128)
  - edges bucketed by (dst tile, src quarter), capacities = max over cores
    rounded up to 128; slot order (block, quarter, tile)
  - gather: int16 idx relative to the quarter, [128, slots/16] replicated
  - S stream: [128, total_chunks*128] bf16, chunk c columns = norm one-hot
    for slots [128c, 128c+128)
"""

import math
import os

import ml_dtypes
import numpy as np

import concourse.bacc as bacc
import concourse.bass as bass
import concourse.mybir as mybir
import concourse.tile as tile
from concourse.bass_utils import run_bass_kernel_spmd

F32 = mybir.dt.float32
BF16 = mybir.dt.bfloat16
I16 = mybir.dt.int16

N_NODES = 100000
N_EDGES = 1600000
N_GRAPHS = 256
D = 128
NCORES = 8
P = 128
NQ = 4

BF = ml_dtypes.bfloat16


QLOC = [4096, 4096, 4096, 256]    # per-core local rows per quarter
QOFF = [0, 4096, 8192, 12288]     # local row offset of each quarter
QROWS = [q * NCORES for q in QLOC]  # table rows per quarter


class Plan:
    """Table layout (quarter-AllGather friendly, unequal quarters):

    node v = c*shard + r with local row r; quarter q = min(r//4096, 3).
    The quarter-q table (a separate DRAM tensor) holds the node at row
    c*QLOC[q] + (r - QOFF[q]), which is exactly the output layout of
    AllGather over each core's h_own_q[q].  Layer l+1's quarter-q
    gathers therefore depend only on quarter-AG q of layer l.  Quarter 3
    is tiny (256 rows/core) so the layer-final AllGather and the gathers
    that consume it are off the critical path.
    """

    def __init__(self, n_nodes, tiles_per_block, g_cap, caps):
        self.n_nodes = n_nodes
        self.nt = caps.shape[0]
        self.shard = self.nt * P
        self.n_pad = self.shard * NCORES
        self.gb = tiles_per_block
        self.nblocks = math.ceil(self.nt / tiles_per_block)
        self.g_cap = g_cap
        self.caps = caps  # [nt, NQ] slot capacities (multiples of 128)

        # slot space ordered by (block, quarter, tile-within-block)
        self.slot_base = np.zeros((self.nt, NQ), np.int64)
        pos = 0
        self.block_info = []  # per block: (chb, [(q, qofs_chunks, nidx)], tiles)
        for b in range(self.nblocks):
            tiles = list(range(b * self.gb, min((b + 1) * self.gb, self.nt)))
            qinfo = []
            chb = 0
            for q in range(NQ):
                nidx = 0
                for t in tiles:
                    self.slot_base[t, q] = pos + nidx
                    nidx += int(self.caps[t, q])
                qinfo.append((q, chb, nidx))
                chb += nidx // P
                pos += nidx
            self.block_info.append((chb, qinfo, tiles))
        self.total_slots = pos
        self.total_chunks = pos // P
        self.max_chb = max(bi[0] for bi in self.block_info)
        self.block_chunk0 = []
        c0 = 0
        for b in range(self.nblocks):
            self.block_chunk0.append(c0)
            c0 += self.block_info[b][0]

    def tile_chunks(self, t):
        """Block-local chunk offsets for dst tile t."""
        b = t // self.gb
        c0 = self.block_chunk0[b]
        out = []
        for (q, qofs, _nidx) in self.block_info[b][1]:
            s0 = int(self.slot_base[t, q])
            nch = int(self.caps[t, q]) // P
            block_slot0 = c0 * P
            for c in range(nch):
                lch = (s0 - block_slot0) // P + c
                out.append(lch)
        return out


def build_plan(col, row, batch, tiles_per_block):
    n = N_NODES
    nt = math.ceil(math.ceil(n / NCORES) / P)
    shard = nt * P
    core = col // shard
    t_local = (col - core * shard) // P
    q = np.minimum((row % shard) // 4096, 3)
    key = (core * nt + t_local) * NQ + q
    counts = np.bincount(key, minlength=NCORES * nt * NQ).reshape(NCORES, nt, NQ)
    caps = counts.max(axis=0)
    caps = np.maximum(((caps + P - 1) // P) * P, P)

    g_cap = 0
    for c in range(NCORES):
        lo = batch[min(c * shard, n - 1)]
        hi = batch[min((c + 1) * shard, n) - 1]
        g_cap = max(g_cap, int(hi - lo + 1))
    g_cap = min(max(((g_cap + 7) // 8) * 8, 16), 128)
    return Plan(n, tiles_per_block, g_cap, caps)


def pack_core_data(plan, c, row, col, normv, norm_self, batch):
    """Pack one core's edge data: gather idx, S stream, s_self, pool."""
    nt, shard = plan.nt, plan.shard
    m = (col >= c * shard) & (col < (c + 1) * shard)
    rowc = row[m]
    colc = col[m]
    nrmc = normv[m]
    t_local = (colc - c * shard) // P
    q = np.minimum((rowc % shard) // 4096, 3)
    dl = colc - (c * shard + t_local * P)

    key = t_local * NQ + q
    order = np.argsort(key, kind="stable")
    skey = key[order]
    grp_start = np.searchsorted(skey, np.arange(nt * NQ), side="left")
    grp_count = np.bincount(skey, minlength=nt * NQ)
    rank = np.arange(len(skey)) - grp_start[skey]
    slot = plan.slot_base.reshape(-1)[skey] + rank

    ns = plan.total_slots
    idx_flat = np.zeros(ns, np.int16)
    qloc_a = np.array(QLOC)[q]
    qoff_a = np.array(QOFF)[q]
    rel = (rowc // shard) * qloc_a + (rowc % shard) - qoff_a
    rel = rel[order]
    assert rel.min() >= 0 and rel.max() < 32768
    idx_flat[slot] = rel.astype(np.int16)
    # Trailing -1 idx would let the SWDGE ucode skip pad descriptors, but the
    # NX-side ring accounting doesn't see the trim and the queue wedges —
    # keep disabled (GCN_TRIM=1 to experiment).
    for b in range(plan.nblocks) if int(os.environ.get("GCN_TRIM", "0")) else []:
        _chb, qinfo, tiles = plan.block_info[b]
        for (qq, _qofs, nidx) in qinfo:
            t_last = tiles[-1]
            fill = int(plan.slot_base[t_last, qq]) + \
                int(grp_count[t_last * NQ + qq])
            seg_end = int(plan.slot_base[tiles[0], qq]) + nidx
            if fill < seg_end:
                idx_flat[fill:seg_end] = -1
    idx16 = np.ascontiguousarray(np.tile(idx_flat.reshape(-1, 16).T, (8, 1)))

    # S stream [128, total_chunks*128] bf16
    s_flat = np.zeros((ns, P), np.float32)
    s_flat[slot, dl[order]] = nrmc[order]
    nch = plan.total_chunks
    s_sb = np.ascontiguousarray(
        s_flat.reshape(nch, P, P).transpose(1, 0, 2).reshape(P, nch * P)
    ).astype(BF)

    # diagonal self-loop S [128, nt*128] bf16
    s_self = np.zeros((P, nt * P), np.float32)
    n_real = min((c + 1) * shard, plan.n_nodes) - c * shard
    nodes = np.arange(n_real)
    s_self[nodes % P, (nodes // P) * P + nodes % P] = norm_self[c * shard + nodes]
    s_self = np.ascontiguousarray(s_self).astype(BF)

    # pooling one-hot [128, nt * g_cap] bf16
    g_lo = int(batch[min(c * shard, plan.n_nodes - 1)])
    pool = np.zeros((P, nt * plan.g_cap), np.float32)
    gl = batch[c * shard + nodes] - g_lo
    pool[nodes % P, (nodes // P) * plan.g_cap + gl] = 1.0
    pool = np.ascontiguousarray(pool).astype(BF)
    return dict(idx16=idx16, s_stream=s_sb, s_self=s_self, pool=pool), g_lo


def build_program(plan):
    nc = bacc.Bacc(num_devices=NCORES,
                   num_swdge_queues=int(os.environ.get("GCN_QUEUES", "4")))
    tch = plan.total_chunks
    nt = plan.nt
    single_packet = bool(int(os.environ.get("GCN_SP", "0")))

    xt_d = [nc.dram_tensor(f"xt{q}", [QROWS[q], D], BF16,
                           kind="ExternalInput") for q in range(NQ)]
    xself_d = nc.dram_tensor("xself", [P, nt * D], BF16, kind="ExternalInput")
    idx_d = nc.dram_tensor("idx16", [P, plan.total_slots // 16], I16,
                           kind="ExternalInput")
    s_d = nc.dram_tensor("s_stream", [P, tch * D], BF16, kind="ExternalInput")
    sself_d = nc.dram_tensor("s_self", [P, nt * D], BF16, kind="ExternalInput")
    pool_d = nc.dram_tensor("pool", [P, nt * plan.g_cap], BF16,
                            kind="ExternalInput")
    # W1 W2 W3 | bias bcast b1 b2 b3 | ones
    wb_d = nc.dram_tensor("wb", [P, 7 * D], BF16, kind="ExternalInput")
    out_d = nc.dram_tensor("pool_out", [3, plan.g_cap, D], F32,
                           kind="ExternalOutput")

    h_own = [nc.dram_tensor(f"h_own{q}", [QLOC[q], D], BF16)
             for q in range(NQ)]
    ag = [[nc.dram_tensor(f"ag{l}_{q}", [QROWS[q], D], BF16,
                          addr_space="Shared") for q in range(NQ)]
          for l in range(2)]

    tables = [[xt_d[q][:] for q in range(NQ)],
              [ag[0][q][:] for q in range(NQ)],
              [ag[1][q][:] for q in range(NQ)]]

    nqueues = nc.num_swdge_queues
    with tile.TileContext(nc) as tc:
        with (
            tc.tile_pool(name="const", bufs=1) as cp,
            tc.tile_pool(name="gpool", bufs=2) as gp,
            tc.tile_pool(name="spool", bufs=2) as spp,
            tc.tile_pool(name="work", bufs=3) as wp,
            tc.tile_pool(name="mt_ps", bufs=4, space="PSUM") as mtp,
            tc.tile_pool(name="h_ps", bufs=2, space="PSUM") as hpp,
            tc.tile_pool(name="pool_ps", bufs=2, space="PSUM") as ppp,
        ):
            idx_sb = cp.tile([P, plan.total_slots // 16], I16)
            nc.sync.dma_start(out=idx_sb[:], in_=idx_d[:])
            sself_sb = cp.tile([P, nt * D], BF16)
            nc.sync.dma_start(out=sself_sb[:], in_=sself_d[:])
            pool_sb = cp.tile([P, nt * plan.g_cap], BF16)
            nc.sync.dma_start(out=pool_sb[:], in_=pool_d[:])
            wb_sb = cp.tile([P, 7 * D], BF16)
            nc.sync.dma_start(out=wb_sb[:], in_=wb_d[:])
            hself = cp.tile([P, nt * D], BF16)
            nc.sync.dma_start(out=hself[:], in_=xself_d[:])
            w_ap = [wb_sb[:, l * D:(l + 1) * D] for l in range(3)]
            brow = [wb_sb[0:1, (3 + l) * D:(4 + l) * D] for l in range(3)]
            ones_row = wb_sb[0:1, 6 * D:7 * D]

            # zero both g buffers once: trailing-trimmed gather rows leave
            # whatever was in SBUF, and uninitialized SBUF may hold NaN/Inf
            # bit patterns that would poison the 0-weighted matmul terms.
            for _i in range(2):
                gz = gp.tile([P, plan.max_chb * D], BF16, tag="g")
                nc.vector.memset(gz[:], 0.0)

            # quarter-AG j can fire once all tiles covering its local rows
            # are written
            ag_after = {}
            for j in range(NQ):
                blk = ((QOFF[j] + QLOC[j] - 1) // P) // plan.gb
                ag_after.setdefault(blk, []).append(j)

            n_layers = int(os.environ.get("GCN_LAYERS", "3"))
            no_ag = bool(os.environ.get("GCN_NO_AG"))
            no_pool = bool(os.environ.get("GCN_NO_POOL"))
            qag = bool(int(os.environ.get("GCN_QAG", "1")))
            for l in range(n_layers):
                pool_ps = ppp.tile([plan.g_cap, D], F32, space="PSUM",
                                   tag="poolps")
                for b in range(plan.nblocks):
                    chb, qinfo, tiles = plan.block_info[b]
                    block_slot0 = plan.block_chunk0[b] * P
                    g = gp.tile([P, plan.max_chb * D], BF16, tag="g")
                    for (q, qofs, nidx) in qinfo:
                        s0 = block_slot0 + qofs * P
                        nc.gpsimd.dma_gather(
                            out_ap=g[:, qofs * D:(qofs + nidx // P) * D]
                                .rearrange("p (c f) -> p c f", f=D),
                            in_ap=tables[l][q],
                            idxs_ap=idx_sb[:, s0 // 16:(s0 + nidx) // 16],
                            num_idxs=nidx,
                            num_idxs_reg=nidx,
                            elem_size=D,
                            single_packet=single_packet,
                            queue_num=q % nqueues,
                        )
                    s = spp.tile([P, plan.max_chb * D], BF16, tag="s")
                    c0 = plan.block_chunk0[b]
                    nc.sync.dma_start(
                        out=s[:, :chb * D],
                        in_=s_d[:, c0 * D:(c0 + chb) * D])
                    for t in tiles:
                        chunks = plan.tile_chunks(t)
                        mt = mtp.tile([P, D], F32, space="PSUM", tag="mt")
                        nc.tensor.matmul(
                            out=mt[:],
                            lhsT=hself[:, t * D:(t + 1) * D],
                            rhs=sself_sb[:, t * D:(t + 1) * D],
                            start=True, stop=False,
                        )
                        for i, lch in enumerate(chunks):
                            nc.tensor.matmul(
                                out=mt[:],
                                lhsT=g[:, lch * D:(lch + 1) * D],
                                rhs=s[:, lch * D:(lch + 1) * D],
                                start=False,
                                stop=(i == len(chunks) - 1),
                            )
                        mts = wp.tile([P, D], BF16, tag="mts")
                        nc.scalar.copy(out=mts[:], in_=mt[:])
                        hp = hpp.tile([P, D], F32, space="PSUM", tag="hps")
                        nc.tensor.matmul(out=hp[:], lhsT=mts[:], rhs=w_ap[l],
                                         start=True, stop=False)
                        nc.tensor.matmul(out=hp[:], lhsT=ones_row,
                                         rhs=brow[l], start=False, stop=True)
                        if l < 2:
                            hb = hself[:, t * D:(t + 1) * D]
                        else:
                            hb_t = wp.tile([P, D], BF16, tag="hb")
                            hb = hb_t[:]
                        nc.scalar.activation(
                            out=hb, in_=hp[:],
                            func=mybir.ActivationFunctionType.Relu)
                        if not no_pool:
                            nc.tensor.matmul(
                                out=pool_ps[:],
                                lhsT=pool_sb[:, t * plan.g_cap:(t + 1) * plan.g_cap],
                                rhs=hb,
                                start=(t == 0),
                                stop=(t == nt - 1),
                            )
                        if l < 2:
                            qt = (t * P) // 4096 if t < 96 else 3
                            nc.sync.dma_start(
                                out=h_own[qt][t * P - QOFF[qt]:
                                              (t + 1) * P - QOFF[qt], :],
                                in_=hb)
                    if qag and l < 2 and not no_ag and b in ag_after:
                        for j in ag_after[b]:
                            nc.gpsimd.collective_compute(
                                "AllGather",
                                mybir.AluOpType.bypass,
                                replica_groups=[list(range(NCORES))],
                                ins=[h_own[j][:]],
                                outs=[ag[l][j][:]],
                            )
                if not qag and l < 2 and not no_ag:
                    for j in range(NQ):
                        nc.gpsimd.collective_compute(
                            "AllGather",
                            mybir.AluOpType.bypass,
                            replica_groups=[list(range(NCORES))],
                            ins=[h_own[j][:]],
                            outs=[ag[l][j][:]],
                        )
                if not no_pool:
                    pc = wp.tile([plan.g_cap, D], F32, tag="poolout")
                    nc.scalar.copy(out=pc[:], in_=pool_ps[:])
                    nc.sync.dma_start(out=out_d[l], in_=pc[:])
    nc.finalize()
    return nc


def kernel(x, edge_index, edge_weight, batch, W1, b1, W2, b2, W3, b3):
    x = np.asarray(x, np.float32)
    edge_index = np.asarray(edge_index, np.int64)
    edge_weight = np.asarray(edge_weight, np.float32)
    batch = np.asarray(batch, np.int64)
    n = x.shape[0]

    row = edge_index[0]
    col = edge_index[1]
    w = edge_weight
    deg = (np.bincount(col, weights=w.astype(np.float64), minlength=n)
           + 1.0)  # self-loop weight 1
    dinv = 1.0 / np.sqrt(deg)
    normv = (dinv[row] * w * dinv[col]).astype(np.float32)
    norm_self = (dinv * dinv).astype(np.float32)

    gb = int(os.environ.get("GCN_GB", "4"))
    plan = build_plan(col, row, batch, gb)
    nc = build_program(plan)

    x_pad = np.zeros((plan.n_pad, D), np.float32)
    x_pad[:n] = x
    x_bf = x_pad.astype(BF)
    # quarter tables (see Plan docstring): xt_q[c*QLOC[q] + rr] = node
    # c*shard + QOFF[q] + rr
    x_tab = []
    for q in range(NQ):
        xq = x_bf.reshape(NCORES, plan.shard, D)[:, QOFF[q]:QOFF[q] + QLOC[q]]
        x_tab.append(np.ascontiguousarray(xq.reshape(QROWS[q], D)))

    wb = np.concatenate(
        [np.asarray(W1, np.float32), np.asarray(W2, np.float32),
         np.asarray(W3, np.float32),
         np.broadcast_to(np.asarray(b1, np.float32), (P, D)),
         np.broadcast_to(np.asarray(b2, np.float32), (P, D)),
         np.broadcast_to(np.asarray(b3, np.float32), (P, D)),
         np.ones((P, D), np.float32)], axis=1)
    wb = np.ascontiguousarray(wb).astype(BF)

    in_maps = []
    g_los = []
    for c in range(NCORES):
        data, g_lo = pack_core_data(plan, c, row, col, normv, norm_self, batch)
        for q in range(NQ):
            data[f"xt{q}"] = x_tab[q]
        xs = x_bf[c * plan.shard:(c + 1) * plan.shard]
        data["xself"] = np.ascontiguousarray(
            xs.reshape(plan.nt, P, D).transpose(1, 0, 2).reshape(P, plan.nt * D))
        data["wb"] = wb
        in_maps.append(data)
        g_los.append(g_lo)

    res = run_bass_kernel_spmd(nc, in_maps, list(range(NCORES)),
                               trace=bool(os.environ.get("GCN_TRACE")))
    global LAST_RESULTS
    LAST_RESULTS = res

    counts = np.maximum(np.bincount(batch, minlength=N_GRAPHS), 1.0)
    embs = []
    for l in range(3):
        acc = np.zeros((N_GRAPHS, D), np.float64)
        for c in range(NCORES):
            part = res.results[c]["pool_out"][l]
            lo = g_los[c]
            hi = min(lo + plan.g_cap, N_GRAPHS)
            acc[lo:hi] += part[:hi - lo]
        embs.append((acc / counts[:, None]).astype(np.float32))
    return tuple(embs)


# revision 29
# speedup vs baseline: 1.0181x; 1.0181x over previous
"""GCN block (3x GCNConv(128,128) + relu + global_mean_pool) on 8 trn2 cores.

v2 strategy (same graph partition by destination node as v1, re-engineered
around the measured bottlenecks: Q7 SWDGE descriptor generation, DVE
tensor_scalar S-builds, and their SBUF-port contention):

  - All device-side tensors are bf16 (PSUM accumulation stays fp32).
  - The one-hot scatter matrices S (static across layers!) are precomputed
    on the HOST in bf16 and streamed per block via HWDGE (nc.sync.dma_start)
    instead of being built per chunk on DVE.  DVE does nothing; pointwise
    work runs on the Scalar/ACT engine, so GpSimd's SWDGE descriptor
    generation no longer contends with DVE 2-port SBUF locks.
  - Self-loops are removed from the gather: each core keeps its own H shard
    resident in SBUF (hself) and applies the self-loop contribution as one
    extra chunk matmul against a host-built diagonal S (s_self).  This cuts
    ~12.5K gather descriptors per core per layer and shrinks bucket padding.
  - The bias is folded into the PE as a rank-1 matmul (ones[1,128]^T @
    b[1,128]) accumulating into the same PSUM as the H@W product, so
    bias+relu collapses into a single ACT activation(Relu) with bf16 cast.
  - Gathers go to 4 SWDGE queues (one per source quarter) so descriptor
    rings drain in parallel; blocks of 4 dst tiles per gather amortize the
    per-instruction overhead.

Per-core data layout (SPMD: same program, per-core arrays):
  - nodes split into 8 shards of 12544 (98 tiles of 128)
  - edges bucketed by (dst tile, src quarter), capacities = max over cores
    rounded up to 128; slot order (block, quarter, tile)
  - gather: int16 idx relative to the quarter, [128, slots/16] replicated
  - S stream: [128, total_chunks*128] bf16, chunk c columns = norm one-hot
    for slots [128c, 128c+128)
"""

import math
import os

import ml_dtypes
import numpy as np

import concourse.bacc as bacc
import concourse.bass as bass
import concourse.mybir as mybir
import concourse.tile as tile
from concourse.bass_utils import run_bass_kernel_spmd

F32 = mybir.dt.float32
BF16 = mybir.dt.bfloat16
I16 = mybir.dt.int16

N_NODES = 100000
N_EDGES = 1600000
N_GRAPHS = 256
D = 128
NCORES = 8
P = 128
NQ = 4

BF = ml_dtypes.bfloat16


QLOC = [4096, 4096, 4096, 256]    # per-core local rows per quarter
QOFF = [0, 4096, 8192, 12288]     # local row offset of each quarter
QROWS = [q * NCORES for q in QLOC]  # table rows per quarter


class Plan:
    """Table layout (quarter-AllGather friendly, unequal quarters):

    node v = c*shard + r with local row r; quarter q = min(r//4096, 3).
    The quarter-q table (a separate DRAM tensor) holds the node at row
    c*QLOC[q] + (r - QOFF[q]), which is exactly the output layout of
    AllGather over each core's h_own_q[q].  Layer l+1's quarter-q
    gathers therefore depend only on quarter-AG q of layer l.  Quarter 3
    is tiny (256 rows/core) so the layer-final AllGather and the gathers
    that consume it are off the critical path.
    """

    def __init__(self, n_nodes, tiles_per_block, g_cap, caps):
        self.n_nodes = n_nodes
        self.nt = caps.shape[0]
        self.shard = self.nt * P
        self.n_pad = self.shard * NCORES
        self.gb = tiles_per_block
        self.nblocks = math.ceil(self.nt / tiles_per_block)
        self.g_cap = g_cap
        self.caps = caps  # [nt, NQ] slot capacities (multiples of 128)

        # slot space ordered by (block, quarter, tile-within-block)
        self.slot_base = np.zeros((self.nt, NQ), np.int64)
        pos = 0
        self.block_info = []  # per block: (chb, [(q, qofs_chunks, nidx)], tiles)
        for b in range(self.nblocks):
            tiles = list(range(b * self.gb, min((b + 1) * self.gb, self.nt)))
            qinfo = []
            chb = 0
            for q in range(NQ):
                nidx = 0
                for t in tiles:
                    self.slot_base[t, q] = pos + nidx
                    nidx += int(self.caps[t, q])
                qinfo.append((q, chb, nidx))
                chb += nidx // P
                pos += nidx
            self.block_info.append((chb, qinfo, tiles))
        self.total_slots = pos
        self.total_chunks = pos // P
        self.max_chb = max(bi[0] for bi in self.block_info)
        self.block_chunk0 = []
        c0 = 0
        for b in range(self.nblocks):
            self.block_chunk0.append(c0)
            c0 += self.block_info[b][0]

    def tile_chunks(self, t):
        """Block-local chunk offsets for dst tile t."""
        b = t // self.gb
        c0 = self.block_chunk0[b]
        out = []
        for (q, qofs, _nidx) in self.block_info[b][1]:
            s0 = int(self.slot_base[t, q])
            nch = int(self.caps[t, q]) // P
            block_slot0 = c0 * P
            for c in range(nch):
                lch = (s0 - block_slot0) // P + c
                out.append(lch)
        return out


def build_plan(col, row, batch, tiles_per_block):
    n = N_NODES
    nt = math.ceil(math.ceil(n / NCORES) / P)
    shard = nt * P
    core = col // shard
    t_local = (col - core * shard) // P
    q = np.minimum((row % shard) // 4096, 3)
    key = (core * nt + t_local) * NQ + q
    counts = np.bincount(key, minlength=NCORES * nt * NQ).reshape(NCORES, nt, NQ)
    caps = counts.max(axis=0)
    caps = np.maximum(((caps + P - 1) // P) * P, P)

    g_cap = 0
    for c in range(NCORES):
        lo = batch[min(c * shard, n - 1)]
        hi = batch[min((c + 1) * shard, n) - 1]
        g_cap = max(g_cap, int(hi - lo + 1))
    g_cap = min(max(((g_cap + 7) // 8) * 8, 16), 128)
    return Plan(n, tiles_per_block, g_cap, caps)


def pack_core_data(plan, c, row, col, normv, norm_self, batch):
    """Pack one core's edge data: gather idx, S stream, s_self, pool."""
    nt, shard = plan.nt, plan.shard
    m = (col >= c * shard) & (col < (c + 1) * shard)
    rowc = row[m]
    colc = col[m]
    nrmc = normv[m]
    t_local = (colc - c * shard) // P
    q = np.minimum((rowc % shard) // 4096, 3)
    dl = colc - (c * shard + t_local * P)

    key = t_local * NQ + q
    order = np.argsort(key, kind="stable")
    skey = key[order]
    grp_start = np.searchsorted(skey, np.arange(nt * NQ), side="left")
    grp_count = np.bincount(skey, minlength=nt * NQ)
    rank = np.arange(len(skey)) - grp_start[skey]
    slot = plan.slot_base.reshape(-1)[skey] + rank

    ns = plan.total_slots
    idx_flat = np.zeros(ns, np.int16)
    qloc_a = np.array(QLOC)[q]
    qoff_a = np.array(QOFF)[q]
    rel = (rowc // shard) * qloc_a + (rowc % shard) - qoff_a
    rel = rel[order]
    assert rel.min() >= 0 and rel.max() < 32768
    idx_flat[slot] = rel.astype(np.int16)
    # Trailing -1 idx would let the SWDGE ucode skip pad descriptors, but the
    # NX-side ring accounting doesn't see the trim and the queue wedges —
    # keep disabled (GCN_TRIM=1 to experiment).
    for b in range(plan.nblocks) if int(os.environ.get("GCN_TRIM", "0")) else []:
        _chb, qinfo, tiles = plan.block_info[b]
        for (qq, _qofs, nidx) in qinfo:
            t_last = tiles[-1]
            fill = int(plan.slot_base[t_last, qq]) + \
                int(grp_count[t_last * NQ + qq])
            seg_end = int(plan.slot_base[tiles[0], qq]) + nidx
            if fill < seg_end:
                idx_flat[fill:seg_end] = -1
    idx16 = np.ascontiguousarray(np.tile(idx_flat.reshape(-1, 16).T, (8, 1)))

    # S stream [128, total_chunks*128] bf16
    s_flat = np.zeros((ns, P), np.float32)
    s_flat[slot, dl[order]] = nrmc[order]
    nch = plan.total_chunks
    s_sb = np.ascontiguousarray(
        s_flat.reshape(nch, P, P).transpose(1, 0, 2).reshape(P, nch * P)
    ).astype(BF)

    # diagonal self-loop S [128, nt*128] bf16
    s_self = np.zeros((P, nt * P), np.float32)
    n_real = min((c + 1) * shard, plan.n_nodes) - c * shard
    nodes = np.arange(n_real)
    s_self[nodes % P, (nodes // P) * P + nodes % P] = norm_self[c * shard + nodes]
    s_self = np.ascontiguousarray(s_self).astype(BF)

    # pooling one-hot [128, nt * g_cap] bf16
    g_lo = int(batch[min(c * shard, plan.n_nodes - 1)])
    pool = np.zeros((P, nt * plan.g_cap), np.float32)
    gl = batch[c * shard + nodes] - g_lo
    pool[nodes % P, (nodes // P) * plan.g_cap + gl] = 1.0
    pool = np.ascontiguousarray(pool).astype(BF)
    return dict(idx16=idx16, s_stream=s_sb, s_self=s_self, pool=pool), g_lo


def build_program(plan):
    nc = bacc.Bacc(num_devices=NCORES,
                   num_swdge_queues=int(os.environ.get("GCN_QUEUES", "4")),
                   dynamic_dma_scratch_size=int(
                       os.environ.get("GCN_SCRATCH", "32768")))
    tch = plan.total_chunks
    nt = plan.nt
    single_packet = bool(int(os.environ.get("GCN_SP", "0")))

    xt_d = [nc.dram_tensor(f"xt{q}", [QROWS[q], D], BF16,
                           kind="ExternalInput") for q in range(NQ)]
    xself_d = nc.dram_tensor("xself", [P, nt * D], BF16, kind="ExternalInput")
    idx_d = nc.dram_tensor("idx16", [P, plan.total_slots // 16], I16,
                           kind="ExternalInput")
    s_d = nc.dram_tensor("s_stream", [P, tch * D], BF16, kind="ExternalInput")
    sself_d = nc.dram_tensor("s_self", [P, nt * D], BF16, kind="ExternalInput")
    pool_d = nc.dram_tensor("pool", [P, nt * plan.g_cap], BF16,
                            kind="ExternalInput")
    # W1 W2 W3 | bias bcast b1 b2 b3 | ones
    wb_d = nc.dram_tensor("wb", [P, 7 * D], BF16, kind="ExternalInput")
    out_d = nc.dram_tensor("pool_out", [3, plan.g_cap, D], F32,
                           kind="ExternalOutput")

    h_own = [nc.dram_tensor(f"h_own{q}", [QLOC[q], D], BF16)
             for q in range(NQ)]
    ag = [[nc.dram_tensor(f"ag{l}_{q}", [QROWS[q], D], BF16,
                          addr_space="Shared") for q in range(NQ)]
          for l in range(2)]

    tables = [[xt_d[q][:] for q in range(NQ)],
              [ag[0][q][:] for q in range(NQ)],
              [ag[1][q][:] for q in range(NQ)]]

    nqueues = nc.num_swdge_queues
    with tile.TileContext(nc) as tc:
        with (
            tc.tile_pool(name="const", bufs=1) as cp,
            tc.tile_pool(name="gpool",
                         bufs=int(os.environ.get("GCN_BUFS", "3"))) as gp,
            tc.tile_pool(name="spool",
                         bufs=int(os.environ.get("GCN_BUFS", "3"))) as spp,
            tc.tile_pool(name="work", bufs=3) as wp,
            tc.tile_pool(name="mt_ps", bufs=4, space="PSUM") as mtp,
            tc.tile_pool(name="h_ps", bufs=2, space="PSUM") as hpp,
            tc.tile_pool(name="pool_ps", bufs=2, space="PSUM") as ppp,
        ):
            idx_sb = cp.tile([P, plan.total_slots // 16], I16)
            nc.sync.dma_start(out=idx_sb[:], in_=idx_d[:])
            sself_sb = cp.tile([P, nt * D], BF16)
            nc.sync.dma_start(out=sself_sb[:], in_=sself_d[:])
            pool_sb = cp.tile([P, nt * plan.g_cap], BF16)
            nc.sync.dma_start(out=pool_sb[:], in_=pool_d[:])
            wb_sb = cp.tile([P, 7 * D], BF16)
            nc.sync.dma_start(out=wb_sb[:], in_=wb_d[:])
            hself = cp.tile([P, nt * D], BF16)
            nc.sync.dma_start(out=hself[:], in_=xself_d[:])
            w_ap = [wb_sb[:, l * D:(l + 1) * D] for l in range(3)]
            brow = [wb_sb[0:1, (3 + l) * D:(4 + l) * D] for l in range(3)]
            ones_row = wb_sb[0:1, 6 * D:7 * D]

            # zero both g buffers once: trailing-trimmed gather rows leave
            # whatever was in SBUF, and uninitialized SBUF may hold NaN/Inf
            # bit patterns that would poison the 0-weighted matmul terms.
            for _i in range(int(os.environ.get("GCN_BUFS", "3"))):
                gz = gp.tile([P, plan.max_chb * D], BF16, tag="g")
                nc.vector.memset(gz[:], 0.0)

            # quarter-AG j can fire once all tiles covering its local rows
            # are written
            ag_after = {}
            for j in range(NQ):
                blk = ((QOFF[j] + QLOC[j] - 1) // P) // plan.gb
                ag_after.setdefault(blk, []).append(j)

            n_layers = int(os.environ.get("GCN_LAYERS", "3"))
            no_ag = bool(os.environ.get("GCN_NO_AG"))
            no_pool = bool(os.environ.get("GCN_NO_POOL"))
            qag = bool(int(os.environ.get("GCN_QAG", "1")))
            for l in range(n_layers):
                pool_ps = ppp.tile([plan.g_cap, D], F32, space="PSUM",
                                   tag="poolps")
                for b in range(plan.nblocks):
                    chb, qinfo, tiles = plan.block_info[b]
                    block_slot0 = plan.block_chunk0[b] * P
                    g = gp.tile([P, plan.max_chb * D], BF16, tag="g")
                    for (q, qofs, nidx) in qinfo:
                        s0 = block_slot0 + qofs * P
                        nc.gpsimd.dma_gather(
                            out_ap=g[:, qofs * D:(qofs + nidx // P) * D]
                                .rearrange("p (c f) -> p c f", f=D),
                            in_ap=tables[l][q],
                            idxs_ap=idx_sb[:, s0 // 16:(s0 + nidx) // 16],
                            num_idxs=nidx,
                            num_idxs_reg=nidx,
                            elem_size=D,
                            single_packet=single_packet,
                            queue_num=q % nqueues,
                        )
                    s = spp.tile([P, plan.max_chb * D], BF16, tag="s")
                    c0 = plan.block_chunk0[b]
                    nc.sync.dma_start(
                        out=s[:, :chb * D],
                        in_=s_d[:, c0 * D:(c0 + chb) * D])
                    for t in tiles:
                        chunks = plan.tile_chunks(t)
                        mt = mtp.tile([P, D], F32, space="PSUM", tag="mt")
                        nc.tensor.matmul(
                            out=mt[:],
                            lhsT=hself[:, t * D:(t + 1) * D],
                            rhs=sself_sb[:, t * D:(t + 1) * D],
                            start=True, stop=False,
                        )
                        for i, lch in enumerate(chunks):
                            nc.tensor.matmul(
                                out=mt[:],
                                lhsT=g[:, lch * D:(lch + 1) * D],
                                rhs=s[:, lch * D:(lch + 1) * D],
                                start=False,
                                stop=(i == len(chunks) - 1),
                            )
                        mts = wp.tile([P, D], BF16, tag="mts")
                        nc.scalar.copy(out=mts[:], in_=mt[:])
                        hp = hpp.tile([P, D], F32, space="PSUM", tag="hps")
                        nc.tensor.matmul(out=hp[:], lhsT=mts[:], rhs=w_ap[l],
                                         start=True, stop=False)
                        nc.tensor.matmul(out=hp[:], lhsT=ones_row,
                                         rhs=brow[l], start=False, stop=True)
                        if l < 2:
                            hb = hself[:, t * D:(t + 1) * D]
                        else:
                            hb_t = wp.tile([P, D], BF16, tag="hb")
                            hb = hb_t[:]
                        nc.scalar.activation(
                            out=hb, in_=hp[:],
                            func=mybir.ActivationFunctionType.Relu)
                        if not no_pool:
                            nc.tensor.matmul(
                                out=pool_ps[:],
                                lhsT=pool_sb[:, t * plan.g_cap:(t + 1) * plan.g_cap],
                                rhs=hb,
                                start=(t == 0),
                                stop=(t == nt - 1),
                            )
                        if l < 2:
                            qt = (t * P) // 4096 if t < 96 else 3
                            nc.sync.dma_start(
                                out=h_own[qt][t * P - QOFF[qt]:
                                              (t + 1) * P - QOFF[qt], :],
                                in_=hb)
                    if qag and l < 2 and not no_ag and b in ag_after:
                        for j in ag_after[b]:
                            nc.gpsimd.collective_compute(
                                "AllGather",
                                mybir.AluOpType.bypass,
                                replica_groups=[list(range(NCORES))],
                                ins=[h_own[j][:]],
                                outs=[ag[l][j][:]],
                            )
                if not qag and l < 2 and not no_ag:
                    for j in range(NQ):
                        nc.gpsimd.collective_compute(
                            "AllGather",
                            mybir.AluOpType.bypass,
                            replica_groups=[list(range(NCORES))],
                            ins=[h_own[j][:]],
                            outs=[ag[l][j][:]],
                        )
                if not no_pool:
                    pc = wp.tile([plan.g_cap, D], F32, tag="poolout")
                    nc.scalar.copy(out=pc[:], in_=pool_ps[:])
                    nc.sync.dma_start(out=out_d[l], in_=pc[:])
    nc.finalize()
    return nc


def kernel(x, edge_index, edge_weight, batch, W1, b1, W2, b2, W3, b3):
    x = np.asarray(x, np.float32)
    edge_index = np.asarray(edge_index, np.int64)
    edge_weight = np.asarray(edge_weight, np.float32)
    batch = np.asarray(batch, np.int64)
    n = x.shape[0]

    row = edge_index[0]
    col = edge_index[1]
    w = edge_weight
    deg = (np.bincount(col, weights=w.astype(np.float64), minlength=n)
           + 1.0)  # self-loop weight 1
    dinv = 1.0 / np.sqrt(deg)
    normv = (dinv[row] * w * dinv[col]).astype(np.float32)
    norm_self = (dinv * dinv).astype(np.float32)

    gb = int(os.environ.get("GCN_GB", "4"))
    plan = build_plan(col, row, batch, gb)
    nc = build_program(plan)

    x_pad = np.zeros((plan.n_pad, D), np.float32)
    x_pad[:n] = x
    x_bf = x_pad.astype(BF)
    # quarter tables (see Plan docstring): xt_q[c*QLOC[q] + rr] = node
    # c*shard + QOFF[q] + rr
    x_tab = []
    for q in range(NQ):
        xq = x_bf.reshape(NCORES, plan.shard, D)[:, QOFF[q]:QOFF[q] + QLOC[q]]
        x_tab.append(np.ascontiguousarray(xq.reshape(QROWS[q], D)))

    wb = np.concatenate(
        [np.asarray(W1, np.float32), np.asarray(W2, np.float32),
         np.asarray(W3, np.float32),
         np.broadcast_to(np.asarray(b1, np.float32), (P, D)),
         np.broadcast_to(np.asarray(b2, np.float32), (P, D)),
         np.broadcast_to(np.asarray(b3, np.float32), (P, D)),
         np.ones((P, D), np.float32)], axis=1)
    wb = np.ascontiguousarray(wb).astype(BF)

    in_maps = []
    g_los = []
    for c in range(NCORES):
        data, g_lo = pack_core_data(plan, c, row, col, normv, norm_self, batch)
        for q in range(NQ):
            data[f"xt{q}"] = x_tab[q]
        xs = x_bf[c * plan.shard:(c + 1) * plan.shard]
        data["xself"] = np.ascontiguousarray(
            xs.reshape(plan.nt, P, D).transpose(1, 0, 2).reshape(P, plan.nt * D))
        data["wb"] = wb
        in_maps.append(data)
        g_los.append(g_lo)

    res = run_bass_kernel_spmd(nc, in_maps, list(range(NCORES)),
                               trace=bool(os.environ.get("GCN_TRACE")))
    global LAST_RESULTS
    LAST_RESULTS = res

    counts = np.maximum(np.bincount(batch, minlength=N_GRAPHS), 1.0)
    embs = []
    for l in range(3):
        acc = np.zeros((N_GRAPHS, D), np.float64)
        for c in range(NCORES):
            part = res.results[c]["pool_out"][l]
            lo = g_los[c]
            hi = min(lo + plan.g_cap, N_GRAPHS)
            acc[lo:hi] += part[:hi - lo]
        embs.append((acc / counts[:, None]).astype(np.float32))
    return tuple(embs)


# revision 32
# speedup vs baseline: 1.6424x; 1.6132x over previous
"""GCN block (3x GCNConv(128,128) + relu + global_mean_pool) on 8 trn2 cores.

v2 strategy (same graph partition by destination node as v1, re-engineered
around the measured bottlenecks: Q7 SWDGE descriptor generation, DVE
tensor_scalar S-builds, and their SBUF-port contention):

  - All device-side tensors are bf16 (PSUM accumulation stays fp32).
  - The one-hot scatter matrices S (static across layers!) are precomputed
    on the HOST in bf16 and streamed per block via HWDGE (nc.sync.dma_start)
    instead of being built per chunk on DVE.  DVE does nothing; pointwise
    work runs on the Scalar/ACT engine, so GpSimd's SWDGE descriptor
    generation no longer contends with DVE 2-port SBUF locks.
  - Self-loops are removed from the gather: each core keeps its own H shard
    resident in SBUF (hself) and applies the self-loop contribution as one
    extra chunk matmul against a host-built diagonal S (s_self).  This cuts
    ~12.5K gather descriptors per core per layer and shrinks bucket padding.
  - The bias is folded into the PE as a rank-1 matmul (ones[1,128]^T @
    b[1,128]) accumulating into the same PSUM as the H@W product, so
    bias+relu collapses into a single ACT activation(Relu) with bf16 cast.
  - Gathers go to 4 SWDGE queues (one per source quarter) so descriptor
    rings drain in parallel; blocks of 4 dst tiles per gather amortize the
    per-instruction overhead.

Per-core data layout (SPMD: same program, per-core arrays):
  - nodes split into 8 shards of 12544 (98 tiles of 128)
  - edges bucketed by (dst tile, src quarter), capacities = max over cores
    rounded up to 128; slot order (block, quarter, tile)
  - gather: int16 idx relative to the quarter, [128, slots/16] replicated
  - S stream: [128, total_chunks*128] bf16, chunk c columns = norm one-hot
    for slots [128c, 128c+128)
"""

import math
import os

import ml_dtypes
import numpy as np

import concourse.bacc as bacc
import concourse.bass as bass
import concourse.mybir as mybir
import concourse.tile as tile
from concourse.bass_utils import run_bass_kernel_spmd

F32 = mybir.dt.float32
BF16 = mybir.dt.bfloat16
I16 = mybir.dt.int16

N_NODES = 100000
N_EDGES = 1600000
N_GRAPHS = 256
D = 128
NCORES = 8
P = 128
NQ = 4

BF = ml_dtypes.bfloat16


if int(os.environ.get("GCN_EQ", "0")):
    QLOC = [3200, 3200, 3200, 2944]
else:
    QLOC = [4096, 4096, 4096, 256]  # per-core local rows per quarter
QOFF = [0] + list(np.cumsum(QLOC)[:-1])  # local row offset of each quarter
QROWS = [q * NCORES for q in QLOC]       # table rows per quarter
QDIV = QLOC[0]                           # quarter = min(r // QDIV, 3)


class Plan:
    """Table layout (quarter-AllGather friendly, unequal quarters):

    node v = c*shard + r with local row r; quarter q = min(r//4096, 3).
    The quarter-q table (a separate DRAM tensor) holds the node at row
    c*QLOC[q] + (r - QOFF[q]), which is exactly the output layout of
    AllGather over each core's h_own_q[q].  Layer l+1's quarter-q
    gathers therefore depend only on quarter-AG q of layer l.  Quarter 3
    is tiny (256 rows/core) so the layer-final AllGather and the gathers
    that consume it are off the critical path.
    """

    def __init__(self, n_nodes, tiles_per_block, g_cap, caps):
        self.n_nodes = n_nodes
        self.nt = caps.shape[0]
        self.shard = self.nt * P
        self.n_pad = self.shard * NCORES
        self.gb = tiles_per_block
        self.nblocks = math.ceil(self.nt / tiles_per_block)
        self.g_cap = g_cap
        self.caps = caps  # [nt, NQ] slot capacities (multiples of 128)

        # slot space ordered by (block, quarter, tile-within-block)
        self.slot_base = np.zeros((self.nt, NQ), np.int64)
        pos = 0
        self.block_info = []  # per block: (chb, [(q, qofs_chunks, nidx)], tiles)
        for b in range(self.nblocks):
            tiles = list(range(b * self.gb, min((b + 1) * self.gb, self.nt)))
            qinfo = []
            chb = 0
            for q in range(NQ):
                nidx = 0
                for t in tiles:
                    self.slot_base[t, q] = pos + nidx
                    nidx += int(self.caps[t, q])
                qinfo.append((q, chb, nidx))
                chb += nidx // P
                pos += nidx
            self.block_info.append((chb, qinfo, tiles))
        self.total_slots = pos
        self.total_chunks = pos // P
        self.max_chb = max(bi[0] for bi in self.block_info)
        self.block_chunk0 = []
        c0 = 0
        for b in range(self.nblocks):
            self.block_chunk0.append(c0)
            c0 += self.block_info[b][0]

    def tile_chunks(self, t):
        """Block-local chunk offsets for dst tile t."""
        b = t // self.gb
        c0 = self.block_chunk0[b]
        out = []
        for (q, qofs, _nidx) in self.block_info[b][1]:
            s0 = int(self.slot_base[t, q])
            nch = int(self.caps[t, q]) // P
            block_slot0 = c0 * P
            for c in range(nch):
                lch = (s0 - block_slot0) // P + c
                out.append(lch)
        return out


def build_plan(col, row, batch, tiles_per_block):
    n = N_NODES
    nt = math.ceil(math.ceil(n / NCORES) / P)
    shard = nt * P
    core = col // shard
    t_local = (col - core * shard) // P
    q = np.minimum((row % shard) // QDIV, 3)
    key = (core * nt + t_local) * NQ + q
    counts = np.bincount(key, minlength=NCORES * nt * NQ).reshape(NCORES, nt, NQ)
    caps = counts.max(axis=0)
    caps = np.maximum(((caps + P - 1) // P) * P, P)

    g_cap = 0
    for c in range(NCORES):
        lo = batch[min(c * shard, n - 1)]
        hi = batch[min((c + 1) * shard, n) - 1]
        g_cap = max(g_cap, int(hi - lo + 1))
    g_cap = min(max(((g_cap + 7) // 8) * 8, 16), 128)
    return Plan(n, tiles_per_block, g_cap, caps)


def pack_core_data(plan, c, row, col, normv, norm_self, batch):
    """Pack one core's edge data: gather idx, S stream, s_self, pool."""
    nt, shard = plan.nt, plan.shard
    m = (col >= c * shard) & (col < (c + 1) * shard)
    rowc = row[m]
    colc = col[m]
    nrmc = normv[m]
    t_local = (colc - c * shard) // P
    q = np.minimum((rowc % shard) // QDIV, 3)
    dl = colc - (c * shard + t_local * P)

    key = t_local * NQ + q
    order = np.argsort(key, kind="stable")
    skey = key[order]
    grp_start = np.searchsorted(skey, np.arange(nt * NQ), side="left")
    grp_count = np.bincount(skey, minlength=nt * NQ)
    rank = np.arange(len(skey)) - grp_start[skey]
    slot = plan.slot_base.reshape(-1)[skey] + rank

    ns = plan.total_slots
    idx_flat = np.zeros(ns, np.int16)
    qloc_a = np.array(QLOC)[q]
    qoff_a = np.array(QOFF)[q]
    rel = (rowc // shard) * qloc_a + (rowc % shard) - qoff_a
    rel = rel[order]
    assert rel.min() >= 0 and rel.max() < 32768
    idx_flat[slot] = rel.astype(np.int16)
    # Trailing -1 idx would let the SWDGE ucode skip pad descriptors, but the
    # NX-side ring accounting doesn't see the trim and the queue wedges —
    # keep disabled (GCN_TRIM=1 to experiment).
    for b in range(plan.nblocks) if int(os.environ.get("GCN_TRIM", "0")) else []:
        _chb, qinfo, tiles = plan.block_info[b]
        for (qq, _qofs, nidx) in qinfo:
            t_last = tiles[-1]
            fill = int(plan.slot_base[t_last, qq]) + \
                int(grp_count[t_last * NQ + qq])
            seg_end = int(plan.slot_base[tiles[0], qq]) + nidx
            if fill < seg_end:
                idx_flat[fill:seg_end] = -1
    idx16 = np.ascontiguousarray(np.tile(idx_flat.reshape(-1, 16).T, (8, 1)))

    # S stream [128, total_chunks*128] bf16
    s_flat = np.zeros((ns, P), np.float32)
    s_flat[slot, dl[order]] = nrmc[order]
    nch = plan.total_chunks
    s_sb = np.ascontiguousarray(
        s_flat.reshape(nch, P, P).transpose(1, 0, 2).reshape(P, nch * P)
    ).astype(BF)

    # diagonal self-loop S [128, nt*128] bf16
    s_self = np.zeros((P, nt * P), np.float32)
    n_real = min((c + 1) * shard, plan.n_nodes) - c * shard
    nodes = np.arange(n_real)
    s_self[nodes % P, (nodes // P) * P + nodes % P] = norm_self[c * shard + nodes]
    s_self = np.ascontiguousarray(s_self).astype(BF)

    # pooling one-hot [128, nt * g_cap] bf16
    g_lo = int(batch[min(c * shard, plan.n_nodes - 1)])
    pool = np.zeros((P, nt * plan.g_cap), np.float32)
    gl = batch[c * shard + nodes] - g_lo
    pool[nodes % P, (nodes // P) * plan.g_cap + gl] = 1.0
    pool = np.ascontiguousarray(pool).astype(BF)
    return dict(idx16=idx16, s_stream=s_sb, s_self=s_self, pool=pool), g_lo


def build_program(plan):
    nc = bacc.Bacc(num_devices=NCORES,
                   num_swdge_queues=int(os.environ.get("GCN_QUEUES", "4")),
                   dynamic_dma_scratch_size=int(
                       os.environ.get("GCN_SCRATCH", "32768")))
    tch = plan.total_chunks
    nt = plan.nt
    single_packet = bool(int(os.environ.get("GCN_SP", "0")))

    xt_d = [nc.dram_tensor(f"xt{q}", [QROWS[q], D], BF16,
                           kind="ExternalInput") for q in range(NQ)]
    xself_d = nc.dram_tensor("xself", [P, nt * D], BF16, kind="ExternalInput")
    idx_d = nc.dram_tensor("idx16", [P, plan.total_slots // 16], I16,
                           kind="ExternalInput")
    s_d = nc.dram_tensor("s_stream", [P, tch * D], BF16, kind="ExternalInput")
    sself_d = nc.dram_tensor("s_self", [P, nt * D], BF16, kind="ExternalInput")
    pool_d = nc.dram_tensor("pool", [P, nt * plan.g_cap], BF16,
                            kind="ExternalInput")
    # W1 W2 W3 | bias bcast b1 b2 b3 | ones
    wb_d = nc.dram_tensor("wb", [P, 7 * D], BF16, kind="ExternalInput")
    out_d = nc.dram_tensor("pool_out", [3, plan.g_cap, D], F32,
                           kind="ExternalOutput")

    h_own = [nc.dram_tensor(f"h_own{q}", [QLOC[q], D], BF16)
             for q in range(NQ)]
    ag = [[nc.dram_tensor(f"ag{l}_{q}", [QROWS[q], D], BF16,
                          addr_space="Shared") for q in range(NQ)]
          for l in range(2)]

    tables = [[xt_d[q][:] for q in range(NQ)],
              [ag[0][q][:] for q in range(NQ)],
              [ag[1][q][:] for q in range(NQ)]]

    nqueues = nc.num_swdge_queues
    with tile.TileContext(nc) as tc:
        with (
            tc.tile_pool(name="const", bufs=1) as cp,
            tc.tile_pool(name="gpool",
                         bufs=int(os.environ.get("GCN_BUFS", "3"))) as gp,
            tc.tile_pool(name="spool",
                         bufs=int(os.environ.get("GCN_BUFS", "3"))) as spp,
            tc.tile_pool(name="work", bufs=3) as wp,
            tc.tile_pool(name="mt_ps", bufs=4, space="PSUM") as mtp,
            tc.tile_pool(name="h_ps", bufs=2, space="PSUM") as hpp,
            tc.tile_pool(name="pool_ps", bufs=2, space="PSUM") as ppp,
        ):
            idx_sb = cp.tile([P, plan.total_slots // 16], I16)
            nc.sync.dma_start(out=idx_sb[:], in_=idx_d[:])
            sself_sb = cp.tile([P, nt * D], BF16)
            nc.sync.dma_start(out=sself_sb[:], in_=sself_d[:])
            pool_sb = cp.tile([P, nt * plan.g_cap], BF16)
            nc.sync.dma_start(out=pool_sb[:], in_=pool_d[:])
            wb_sb = cp.tile([P, 7 * D], BF16)
            nc.sync.dma_start(out=wb_sb[:], in_=wb_d[:])
            hself = cp.tile([P, nt * D], BF16)
            nc.sync.dma_start(out=hself[:], in_=xself_d[:])
            w_ap = [wb_sb[:, l * D:(l + 1) * D] for l in range(3)]
            brow = [wb_sb[0:1, (3 + l) * D:(4 + l) * D] for l in range(3)]
            ones_row = wb_sb[0:1, 6 * D:7 * D]

            # zero both g buffers once: trailing-trimmed gather rows leave
            # whatever was in SBUF, and uninitialized SBUF may hold NaN/Inf
            # bit patterns that would poison the 0-weighted matmul terms.
            for _i in range(int(os.environ.get("GCN_BUFS", "3"))):
                gz = gp.tile([P, plan.max_chb * D], BF16, tag="g")
                nc.vector.memset(gz[:], 0.0)

            # quarter-AG j can fire once all tiles covering its local rows
            # are written
            ag_after = {}
            for j in range(NQ):
                blk = ((QOFF[j] + QLOC[j] - 1) // P) // plan.gb
                ag_after.setdefault(blk, []).append(j)

            n_layers = int(os.environ.get("GCN_LAYERS", "3"))
            no_ag = bool(os.environ.get("GCN_NO_AG"))
            no_pool = bool(os.environ.get("GCN_NO_POOL"))
            qag = bool(int(os.environ.get("GCN_QAG", "1")))
            for l in range(n_layers):
                pool_ps = ppp.tile([plan.g_cap, D], F32, space="PSUM",
                                   tag="poolps")
                for b in range(plan.nblocks):
                    chb, qinfo, tiles = plan.block_info[b]
                    block_slot0 = plan.block_chunk0[b] * P
                    g = gp.tile([P, plan.max_chb * D], BF16, tag="g")
                    for (q, qofs, nidx) in qinfo:
                        s0 = block_slot0 + qofs * P
                        nc.gpsimd.dma_gather(
                            out_ap=g[:, qofs * D:(qofs + nidx // P) * D]
                                .rearrange("p (c f) -> p c f", f=D),
                            in_ap=tables[l][q],
                            idxs_ap=idx_sb[:, s0 // 16:(s0 + nidx) // 16],
                            num_idxs=nidx,
                            num_idxs_reg=nidx,
                            elem_size=D,
                            single_packet=single_packet,
                            queue_num=q % nqueues,
                        )
                    s = spp.tile([P, plan.max_chb * D], BF16, tag="s")
                    c0 = plan.block_chunk0[b]
                    nc.sync.dma_start(
                        out=s[:, :chb * D],
                        in_=s_d[:, c0 * D:(c0 + chb) * D])
                    for t in tiles:
                        chunks = plan.tile_chunks(t)
                        mt = mtp.tile([P, D], F32, space="PSUM", tag="mt")
                        nc.tensor.matmul(
                            out=mt[:],
                            lhsT=hself[:, t * D:(t + 1) * D],
                            rhs=sself_sb[:, t * D:(t + 1) * D],
                            start=True, stop=False,
                        )
                        for i, lch in enumerate(chunks):
                            nc.tensor.matmul(
                                out=mt[:],
                                lhsT=g[:, lch * D:(lch + 1) * D],
                                rhs=s[:, lch * D:(lch + 1) * D],
                                start=False,
                                stop=(i == len(chunks) - 1),
                            )
                        mts = wp.tile([P, D], BF16, tag="mts")
                        nc.scalar.copy(out=mts[:], in_=mt[:])
                        hp = hpp.tile([P, D], F32, space="PSUM", tag="hps")
                        nc.tensor.matmul(out=hp[:], lhsT=mts[:], rhs=w_ap[l],
                                         start=True, stop=False)
                        nc.tensor.matmul(out=hp[:], lhsT=ones_row,
                                         rhs=brow[l], start=False, stop=True)
                        if l < 2:
                            hb = hself[:, t * D:(t + 1) * D]
                        else:
                            hb_t = wp.tile([P, D], BF16, tag="hb")
                            hb = hb_t[:]
                        nc.scalar.activation(
                            out=hb, in_=hp[:],
                            func=mybir.ActivationFunctionType.Relu)
                        if not no_pool:
                            nc.tensor.matmul(
                                out=pool_ps[:],
                                lhsT=pool_sb[:, t * plan.g_cap:(t + 1) * plan.g_cap],
                                rhs=hb,
                                start=(t == 0),
                                stop=(t == nt - 1),
                            )
                        if l < 2:
                            qt = min((t * P) // QDIV, 3)
                            nc.sync.dma_start(
                                out=h_own[qt][t * P - QOFF[qt]:
                                              (t + 1) * P - QOFF[qt], :],
                                in_=hb)
                    if qag and l < 2 and not no_ag and b in ag_after:
                        for j in ag_after[b]:
                            nc.gpsimd.collective_compute(
                                "AllGather",
                                mybir.AluOpType.bypass,
                                replica_groups=[list(range(NCORES))],
                                ins=[h_own[j][:]],
                                outs=[ag[l][j][:]],
                            )
                if not qag and l < 2 and not no_ag:
                    for j in range(NQ):
                        nc.gpsimd.collective_compute(
                            "AllGather",
                            mybir.AluOpType.bypass,
                            replica_groups=[list(range(NCORES))],
                            ins=[h_own[j][:]],
                            outs=[ag[l][j][:]],
                        )
                if not no_pool:
                    pc = wp.tile([plan.g_cap, D], F32, tag="poolout")
                    nc.scalar.copy(out=pc[:], in_=pool_ps[:])
                    nc.sync.dma_start(out=out_d[l], in_=pc[:])
    nc.finalize()
    return nc


def kernel(x, edge_index, edge_weight, batch, W1, b1, W2, b2, W3, b3):
    x = np.asarray(x, np.float32)
    edge_index = np.asarray(edge_index, np.int64)
    edge_weight = np.asarray(edge_weight, np.float32)
    batch = np.asarray(batch, np.int64)
    n = x.shape[0]

    row = edge_index[0]
    col = edge_index[1]
    w = edge_weight
    deg = (np.bincount(col, weights=w.astype(np.float64), minlength=n)
           + 1.0)  # self-loop weight 1
    dinv = 1.0 / np.sqrt(deg)
    normv = (dinv[row] * w * dinv[col]).astype(np.float32)
    norm_self = (dinv * dinv).astype(np.float32)

    gb = int(os.environ.get("GCN_GB", "4"))
    plan = build_plan(col, row, batch, gb)
    nc = build_program(plan)

    x_pad = np.zeros((plan.n_pad, D), np.float32)
    x_pad[:n] = x
    x_bf = x_pad.astype(BF)
    # quarter tables (see Plan docstring): xt_q[c*QLOC[q] + rr] = node
    # c*shard + QOFF[q] + rr
    x_tab = []
    for q in range(NQ):
        xq = x_bf.reshape(NCORES, plan.shard, D)[:, QOFF[q]:QOFF[q] + QLOC[q]]
        x_tab.append(np.ascontiguousarray(xq.reshape(QROWS[q], D)))

    wb = np.concatenate(
        [np.asarray(W1, np.float32), np.asarray(W2, np.float32),
         np.asarray(W3, np.float32),
         np.broadcast_to(np.asarray(b1, np.float32), (P, D)),
         np.broadcast_to(np.asarray(b2, np.float32), (P, D)),
         np.broadcast_to(np.asarray(b3, np.float32), (P, D)),
         np.ones((P, D), np.float32)], axis=1)
    wb = np.ascontiguousarray(wb).astype(BF)

    in_maps = []
    g_los = []
    for c in range(NCORES):
        data, g_lo = pack_core_data(plan, c, row, col, normv, norm_self, batch)
        for q in range(NQ):
            data[f"xt{q}"] = x_tab[q]
        xs = x_bf[c * plan.shard:(c + 1) * plan.shard]
        data["xself"] = np.ascontiguousarray(
            xs.reshape(plan.nt, P, D).transpose(1, 0, 2).reshape(P, plan.nt * D))
        data["wb"] = wb
        in_maps.append(data)
        g_los.append(g_lo)

    res = run_bass_kernel_spmd(nc, in_maps, list(range(NCORES)),
                               trace=bool(os.environ.get("GCN_TRACE")))
    global LAST_RESULTS
    LAST_RESULTS = res

    counts = np.maximum(np.bincount(batch, minlength=N_GRAPHS), 1.0)
    embs = []
    for l in range(3):
        acc = np.zeros((N_GRAPHS, D), np.float64)
        for c in range(NCORES):
            part = res.results[c]["pool_out"][l]
            lo = g_los[c]
            hi = min(lo + plan.g_cap, N_GRAPHS)
            acc[lo:hi] += part[:hi - lo]
        embs.append((acc / counts[:, None]).astype(np.float32))
    return tuple(embs)


# revision 34
# speedup vs baseline: 1.7581x; 1.0705x over previous
"""GCN block (3x GCNConv(128,128) + relu + global_mean_pool) on 8 trn2 cores.

v2 strategy (same graph partition by destination node as v1, re-engineered
around the measured bottlenecks: Q7 SWDGE descriptor generation, DVE
tensor_scalar S-builds, and their SBUF-port contention):

  - All device-side tensors are bf16 (PSUM accumulation stays fp32).
  - The one-hot scatter matrices S (static across layers!) are precomputed
    on the HOST in bf16 and streamed per block via HWDGE (nc.sync.dma_start)
    instead of being built per chunk on DVE.  DVE does nothing; pointwise
    work runs on the Scalar/ACT engine, so GpSimd's SWDGE descriptor
    generation no longer contends with DVE 2-port SBUF locks.
  - Self-loops are removed from the gather: each core keeps its own H shard
    resident in SBUF (hself) and applies the self-loop contribution as one
    extra chunk matmul against a host-built diagonal S (s_self).  This cuts
    ~12.5K gather descriptors per core per layer and shrinks bucket padding.
  - The bias is folded into the PE as a rank-1 matmul (ones[1,128]^T @
    b[1,128]) accumulating into the same PSUM as the H@W product, so
    bias+relu collapses into a single ACT activation(Relu) with bf16 cast.
  - Gathers go to 4 SWDGE queues (one per source quarter) so descriptor
    rings drain in parallel; blocks of 4 dst tiles per gather amortize the
    per-instruction overhead.

Per-core data layout (SPMD: same program, per-core arrays):
  - nodes split into 8 shards of 12544 (98 tiles of 128)
  - edges bucketed by (dst tile, src quarter), capacities = max over cores
    rounded up to 128; slot order (block, quarter, tile)
  - gather: int16 idx relative to the quarter, [128, slots/16] replicated
  - S stream: [128, total_chunks*128] bf16, chunk c columns = norm one-hot
    for slots [128c, 128c+128)
"""

import math
import os

import ml_dtypes
import numpy as np

import concourse.bacc as bacc
import concourse.bass as bass
import concourse.mybir as mybir
import concourse.tile as tile
from concourse.bass_utils import run_bass_kernel_spmd

F32 = mybir.dt.float32
BF16 = mybir.dt.bfloat16
I16 = mybir.dt.int16

N_NODES = 100000
N_EDGES = 1600000
N_GRAPHS = 256
D = 128
NCORES = 8
P = 128
NQ = 4

BF = ml_dtypes.bfloat16


if int(os.environ.get("GCN_UNEQ", "0")):
    QLOC = [4096, 4096, 4096, 256]  # experiment: tiny final quarter
else:
    QLOC = [3200, 3200, 3200, 2944]  # per-core local rows per quarter
QOFF = [0] + list(np.cumsum(QLOC)[:-1])  # local row offset of each quarter
QROWS = [q * NCORES for q in QLOC]       # table rows per quarter
QDIV = QLOC[0]                           # quarter = min(r // QDIV, 3)


class Plan:
    """Table layout (quarter-AllGather friendly, unequal quarters):

    node v = c*shard + r with local row r; quarter q = min(r//QDIV, 3).
    The quarter-q table (a separate DRAM tensor) holds the node at row
    c*QLOC[q] + (r - QOFF[q]), which is exactly the output layout of
    AllGather over each core's h_own[q].  Layer l+1's quarter-q gathers
    therefore depend only on quarter-AG q of layer l, so the four
    AllGathers pipeline under the tail of each layer's compute.
    Boundaries are tile-aligned (multiples of 128).
    """

    def __init__(self, n_nodes, tiles_per_block, g_cap, caps):
        self.n_nodes = n_nodes
        self.nt = caps.shape[0]
        self.shard = self.nt * P
        self.n_pad = self.shard * NCORES
        self.gb = tiles_per_block
        self.nblocks = math.ceil(self.nt / tiles_per_block)
        self.g_cap = g_cap
        self.caps = caps  # [nt, NQ] slot capacities (multiples of 128)

        # slot space ordered by (block, quarter, tile-within-block)
        self.slot_base = np.zeros((self.nt, NQ), np.int64)
        pos = 0
        self.block_info = []  # per block: (chb, [(q, qofs_chunks, nidx)], tiles)
        for b in range(self.nblocks):
            tiles = list(range(b * self.gb, min((b + 1) * self.gb, self.nt)))
            qinfo = []
            chb = 0
            for q in range(NQ):
                nidx = 0
                for t in tiles:
                    self.slot_base[t, q] = pos + nidx
                    nidx += int(self.caps[t, q])
                qinfo.append((q, chb, nidx))
                chb += nidx // P
                pos += nidx
            self.block_info.append((chb, qinfo, tiles))
        self.total_slots = pos
        self.total_chunks = pos // P
        self.max_chb = max(bi[0] for bi in self.block_info)
        self.block_chunk0 = []
        c0 = 0
        for b in range(self.nblocks):
            self.block_chunk0.append(c0)
            c0 += self.block_info[b][0]

    def tile_chunks(self, t):
        """Block-local chunk offsets for dst tile t."""
        b = t // self.gb
        c0 = self.block_chunk0[b]
        out = []
        for (q, qofs, _nidx) in self.block_info[b][1]:
            s0 = int(self.slot_base[t, q])
            nch = int(self.caps[t, q]) // P
            block_slot0 = c0 * P
            for c in range(nch):
                lch = (s0 - block_slot0) // P + c
                out.append(lch)
        return out


def build_plan(col, row, batch, tiles_per_block):
    n = N_NODES
    nt = math.ceil(math.ceil(n / NCORES) / P)
    shard = nt * P
    core = col // shard
    t_local = (col - core * shard) // P
    q = np.minimum((row % shard) // QDIV, 3)
    key = (core * nt + t_local) * NQ + q
    counts = np.bincount(key, minlength=NCORES * nt * NQ).reshape(NCORES, nt, NQ)
    caps = counts.max(axis=0)
    caps = np.maximum(((caps + P - 1) // P) * P, P)

    g_cap = 0
    for c in range(NCORES):
        lo = batch[min(c * shard, n - 1)]
        hi = batch[min((c + 1) * shard, n) - 1]
        g_cap = max(g_cap, int(hi - lo + 1))
    g_cap = min(max(((g_cap + 7) // 8) * 8, 16), 128)
    return Plan(n, tiles_per_block, g_cap, caps)


def pack_core_data(plan, c, row, col, normv, norm_self, batch):
    """Pack one core's edge data: gather idx, S stream, s_self, pool."""
    nt, shard = plan.nt, plan.shard
    m = (col >= c * shard) & (col < (c + 1) * shard)
    rowc = row[m]
    colc = col[m]
    nrmc = normv[m]
    t_local = (colc - c * shard) // P
    q = np.minimum((rowc % shard) // QDIV, 3)
    dl = colc - (c * shard + t_local * P)

    key = t_local * NQ + q
    order = np.argsort(key, kind="stable")
    skey = key[order]
    grp_start = np.searchsorted(skey, np.arange(nt * NQ), side="left")
    grp_count = np.bincount(skey, minlength=nt * NQ)
    rank = np.arange(len(skey)) - grp_start[skey]
    slot = plan.slot_base.reshape(-1)[skey] + rank

    ns = plan.total_slots
    idx_flat = np.zeros(ns, np.int16)
    qloc_a = np.array(QLOC)[q]
    qoff_a = np.array(QOFF)[q]
    rel = (rowc // shard) * qloc_a + (rowc % shard) - qoff_a
    rel = rel[order]
    assert rel.min() >= 0 and rel.max() < 32768
    idx_flat[slot] = rel.astype(np.int16)
    # Trailing -1 idx would let the SWDGE ucode skip pad descriptors, but the
    # NX-side ring accounting doesn't see the trim and the queue wedges —
    # keep disabled (GCN_TRIM=1 to experiment).
    for b in range(plan.nblocks) if int(os.environ.get("GCN_TRIM", "0")) else []:
        _chb, qinfo, tiles = plan.block_info[b]
        for (qq, _qofs, nidx) in qinfo:
            t_last = tiles[-1]
            fill = int(plan.slot_base[t_last, qq]) + \
                int(grp_count[t_last * NQ + qq])
            seg_end = int(plan.slot_base[tiles[0], qq]) + nidx
            if fill < seg_end:
                idx_flat[fill:seg_end] = -1
    idx16 = np.ascontiguousarray(np.tile(idx_flat.reshape(-1, 16).T, (8, 1)))

    # S stream [128, total_chunks*128] bf16
    s_flat = np.zeros((ns, P), np.float32)
    s_flat[slot, dl[order]] = nrmc[order]
    nch = plan.total_chunks
    s_sb = np.ascontiguousarray(
        s_flat.reshape(nch, P, P).transpose(1, 0, 2).reshape(P, nch * P)
    ).astype(BF)

    # diagonal self-loop S [128, nt*128] bf16
    s_self = np.zeros((P, nt * P), np.float32)
    n_real = min((c + 1) * shard, plan.n_nodes) - c * shard
    nodes = np.arange(n_real)
    s_self[nodes % P, (nodes // P) * P + nodes % P] = norm_self[c * shard + nodes]
    s_self = np.ascontiguousarray(s_self).astype(BF)

    # pooling one-hot [128, nt * g_cap] bf16
    g_lo = int(batch[min(c * shard, plan.n_nodes - 1)])
    pool = np.zeros((P, nt * plan.g_cap), np.float32)
    gl = batch[c * shard + nodes] - g_lo
    pool[nodes % P, (nodes // P) * plan.g_cap + gl] = 1.0
    pool = np.ascontiguousarray(pool).astype(BF)
    return dict(idx16=idx16, s_stream=s_sb, s_self=s_self, pool=pool), g_lo


def build_program(plan):
    nc = bacc.Bacc(num_devices=NCORES,
                   num_swdge_queues=int(os.environ.get("GCN_QUEUES", "4")),
                   dynamic_dma_scratch_size=int(
                       os.environ.get("GCN_SCRATCH", "16384")))
    tch = plan.total_chunks
    nt = plan.nt
    single_packet = bool(int(os.environ.get("GCN_SP", "0")))

    xt_d = [nc.dram_tensor(f"xt{q}", [QROWS[q], D], BF16,
                           kind="ExternalInput") for q in range(NQ)]
    xself_d = nc.dram_tensor("xself", [P, nt * D], BF16, kind="ExternalInput")
    idx_d = nc.dram_tensor("idx16", [P, plan.total_slots // 16], I16,
                           kind="ExternalInput")
    s_d = nc.dram_tensor("s_stream", [P, tch * D], BF16, kind="ExternalInput")
    sself_d = nc.dram_tensor("s_self", [P, nt * D], BF16, kind="ExternalInput")
    pool_d = nc.dram_tensor("pool", [P, nt * plan.g_cap], BF16,
                            kind="ExternalInput")
    # W1 W2 W3 | bias bcast b1 b2 b3 | ones
    wb_d = nc.dram_tensor("wb", [P, 7 * D], BF16, kind="ExternalInput")
    out_d = nc.dram_tensor("pool_out", [3, plan.g_cap, D], F32,
                           kind="ExternalOutput")

    h_own = [nc.dram_tensor(f"h_own{q}", [QLOC[q], D], BF16)
             for q in range(NQ)]
    ag = [[nc.dram_tensor(f"ag{l}_{q}", [QROWS[q], D], BF16,
                          addr_space="Shared") for q in range(NQ)]
          for l in range(2)]

    tables = [[xt_d[q][:] for q in range(NQ)],
              [ag[0][q][:] for q in range(NQ)],
              [ag[1][q][:] for q in range(NQ)]]

    nqueues = nc.num_swdge_queues
    with tile.TileContext(nc) as tc:
        with (
            tc.tile_pool(name="const", bufs=1) as cp,
            tc.tile_pool(name="gpool",
                         bufs=int(os.environ.get("GCN_BUFS", "2"))) as gp,
            tc.tile_pool(name="spool",
                         bufs=int(os.environ.get("GCN_BUFS", "2"))) as spp,
            tc.tile_pool(name="work", bufs=3) as wp,
            tc.tile_pool(name="mt_ps", bufs=4, space="PSUM") as mtp,
            tc.tile_pool(name="h_ps", bufs=2, space="PSUM") as hpp,
            tc.tile_pool(name="pool_ps", bufs=2, space="PSUM") as ppp,
        ):
            idx_sb = cp.tile([P, plan.total_slots // 16], I16)
            nc.sync.dma_start(out=idx_sb[:], in_=idx_d[:])
            sself_sb = cp.tile([P, nt * D], BF16)
            nc.sync.dma_start(out=sself_sb[:], in_=sself_d[:])
            pool_sb = cp.tile([P, nt * plan.g_cap], BF16)
            nc.sync.dma_start(out=pool_sb[:], in_=pool_d[:])
            wb_sb = cp.tile([P, 7 * D], BF16)
            nc.sync.dma_start(out=wb_sb[:], in_=wb_d[:])
            hself = cp.tile([P, nt * D], BF16)
            nc.sync.dma_start(out=hself[:], in_=xself_d[:])
            w_ap = [wb_sb[:, l * D:(l + 1) * D] for l in range(3)]
            brow = [wb_sb[0:1, (3 + l) * D:(4 + l) * D] for l in range(3)]
            ones_row = wb_sb[0:1, 6 * D:7 * D]

            # zero both g buffers once: trailing-trimmed gather rows leave
            # whatever was in SBUF, and uninitialized SBUF may hold NaN/Inf
            # bit patterns that would poison the 0-weighted matmul terms.
            for _i in range(int(os.environ.get("GCN_BUFS", "2"))):
                gz = gp.tile([P, plan.max_chb * D], BF16, tag="g")
                nc.vector.memset(gz[:], 0.0)

            # quarter-AG j can fire once all tiles covering its local rows
            # are written
            ag_after = {}
            for j in range(NQ):
                blk = ((QOFF[j] + QLOC[j] - 1) // P) // plan.gb
                ag_after.setdefault(blk, []).append(j)

            n_layers = int(os.environ.get("GCN_LAYERS", "3"))
            no_ag = bool(os.environ.get("GCN_NO_AG"))
            no_pool = bool(os.environ.get("GCN_NO_POOL"))
            qag = bool(int(os.environ.get("GCN_QAG", "1")))
            for l in range(n_layers):
                pool_ps = ppp.tile([plan.g_cap, D], F32, space="PSUM",
                                   tag="poolps")
                for b in range(plan.nblocks):
                    chb, qinfo, tiles = plan.block_info[b]
                    block_slot0 = plan.block_chunk0[b] * P
                    g = gp.tile([P, plan.max_chb * D], BF16, tag="g")
                    for (q, qofs, nidx) in qinfo:
                        s0 = block_slot0 + qofs * P
                        nc.gpsimd.dma_gather(
                            out_ap=g[:, qofs * D:(qofs + nidx // P) * D]
                                .rearrange("p (c f) -> p c f", f=D),
                            in_ap=tables[l][q],
                            idxs_ap=idx_sb[:, s0 // 16:(s0 + nidx) // 16],
                            num_idxs=nidx,
                            num_idxs_reg=nidx,
                            elem_size=D,
                            single_packet=single_packet,
                            queue_num=q % nqueues,
                        )
                    s = spp.tile([P, plan.max_chb * D], BF16, tag="s")
                    c0 = plan.block_chunk0[b]
                    nc.sync.dma_start(
                        out=s[:, :chb * D],
                        in_=s_d[:, c0 * D:(c0 + chb) * D])
                    for t in tiles:
                        chunks = plan.tile_chunks(t)
                        mt = mtp.tile([P, D], F32, space="PSUM", tag="mt")
                        nc.tensor.matmul(
                            out=mt[:],
                            lhsT=hself[:, t * D:(t + 1) * D],
                            rhs=sself_sb[:, t * D:(t + 1) * D],
                            start=True, stop=False,
                        )
                        for i, lch in enumerate(chunks):
                            nc.tensor.matmul(
                                out=mt[:],
                                lhsT=g[:, lch * D:(lch + 1) * D],
                                rhs=s[:, lch * D:(lch + 1) * D],
                                start=False,
                                stop=(i == len(chunks) - 1),
                            )
                        mts = wp.tile([P, D], BF16, tag="mts")
                        nc.scalar.copy(out=mts[:], in_=mt[:])
                        hp = hpp.tile([P, D], F32, space="PSUM", tag="hps")
                        nc.tensor.matmul(out=hp[:], lhsT=mts[:], rhs=w_ap[l],
                                         start=True, stop=False)
                        nc.tensor.matmul(out=hp[:], lhsT=ones_row,
                                         rhs=brow[l], start=False, stop=True)
                        if l < 2:
                            hb = hself[:, t * D:(t + 1) * D]
                        else:
                            hb_t = wp.tile([P, D], BF16, tag="hb")
                            hb = hb_t[:]
                        nc.scalar.activation(
                            out=hb, in_=hp[:],
                            func=mybir.ActivationFunctionType.Relu)
                        if not no_pool:
                            nc.tensor.matmul(
                                out=pool_ps[:],
                                lhsT=pool_sb[:, t * plan.g_cap:(t + 1) * plan.g_cap],
                                rhs=hb,
                                start=(t == 0),
                                stop=(t == nt - 1),
                            )
                        if l < 2:
                            qt = min((t * P) // QDIV, 3)
                            nc.sync.dma_start(
                                out=h_own[qt][t * P - QOFF[qt]:
                                              (t + 1) * P - QOFF[qt], :],
                                in_=hb)
                    if qag and l < 2 and not no_ag and b in ag_after:
                        for j in ag_after[b]:
                            nc.gpsimd.collective_compute(
                                "AllGather",
                                mybir.AluOpType.bypass,
                                replica_groups=[list(range(NCORES))],
                                ins=[h_own[j][:]],
                                outs=[ag[l][j][:]],
                            )
                if not qag and l < 2 and not no_ag:
                    for j in range(NQ):
                        nc.gpsimd.collective_compute(
                            "AllGather",
                            mybir.AluOpType.bypass,
                            replica_groups=[list(range(NCORES))],
                            ins=[h_own[j][:]],
                            outs=[ag[l][j][:]],
                        )
                if not no_pool:
                    pc = wp.tile([plan.g_cap, D], F32, tag="poolout")
                    nc.scalar.copy(out=pc[:], in_=pool_ps[:])
                    nc.sync.dma_start(out=out_d[l], in_=pc[:])
    nc.finalize()
    return nc


def kernel(x, edge_index, edge_weight, batch, W1, b1, W2, b2, W3, b3):
    x = np.asarray(x, np.float32)
    edge_index = np.asarray(edge_index, np.int64)
    edge_weight = np.asarray(edge_weight, np.float32)
    batch = np.asarray(batch, np.int64)
    n = x.shape[0]

    row = edge_index[0]
    col = edge_index[1]
    w = edge_weight
    deg = (np.bincount(col, weights=w.astype(np.float64), minlength=n)
           + 1.0)  # self-loop weight 1
    dinv = 1.0 / np.sqrt(deg)
    normv = (dinv[row] * w * dinv[col]).astype(np.float32)
    norm_self = (dinv * dinv).astype(np.float32)

    gb = int(os.environ.get("GCN_GB", "4"))
    plan = build_plan(col, row, batch, gb)
    nc = build_program(plan)

    x_pad = np.zeros((plan.n_pad, D), np.float32)
    x_pad[:n] = x
    x_bf = x_pad.astype(BF)
    # quarter tables (see Plan docstring): xt_q[c*QLOC[q] + rr] = node
    # c*shard + QOFF[q] + rr
    x_tab = []
    for q in range(NQ):
        xq = x_bf.reshape(NCORES, plan.shard, D)[:, QOFF[q]:QOFF[q] + QLOC[q]]
        x_tab.append(np.ascontiguousarray(xq.reshape(QROWS[q], D)))

    wb = np.concatenate(
        [np.asarray(W1, np.float32), np.asarray(W2, np.float32),
         np.asarray(W3, np.float32),
         np.broadcast_to(np.asarray(b1, np.float32), (P, D)),
         np.broadcast_to(np.asarray(b2, np.float32), (P, D)),
         np.broadcast_to(np.asarray(b3, np.float32), (P, D)),
         np.ones((P, D), np.float32)], axis=1)
    wb = np.ascontiguousarray(wb).astype(BF)

    in_maps = []
    g_los = []
    for c in range(NCORES):
        data, g_lo = pack_core_data(plan, c, row, col, normv, norm_self, batch)
        for q in range(NQ):
            data[f"xt{q}"] = x_tab[q]
        xs = x_bf[c * plan.shard:(c + 1) * plan.shard]
        data["xself"] = np.ascontiguousarray(
            xs.reshape(plan.nt, P, D).transpose(1, 0, 2).reshape(P, plan.nt * D))
        data["wb"] = wb
        in_maps.append(data)
        g_los.append(g_lo)

    res = run_bass_kernel_spmd(nc, in_maps, list(range(NCORES)),
                               trace=bool(os.environ.get("GCN_TRACE")))
    global LAST_RESULTS
    LAST_RESULTS = res

    counts = np.maximum(np.bincount(batch, minlength=N_GRAPHS), 1.0)
    embs = []
    for l in range(3):
        acc = np.zeros((N_GRAPHS, D), np.float64)
        for c in range(NCORES):
            part = res.results[c]["pool_out"][l]
            lo = g_los[c]
            hi = min(lo + plan.g_cap, N_GRAPHS)
            acc[lo:hi] += part[:hi - lo]
        embs.append((acc / counts[:, None]).astype(np.float32))
    return tuple(embs)
